# revision 17
# baseline (speedup 1.0000x reference)
"""Paged-attention GPT-2 decode kernel for 8 Trainium2 NeuronCores.

Sharding: tensor-parallel across heads (Megatron) — 2 heads per core.
Each core gets its head-pair slice of w_attn / w_proj / KV caches and
computes a partial [32,1024] c_proj output; host sums the 8 partials.

The program is specialized to the observed context_lens/block_tables.
Host-side prep packs each sequence's cached K (transposed, [128hd x T])
and V (token-major, with a fused ones-column for the softmax
denominator) into bf16 "SBUF images" so the kernel needs only ONE big
DMA per sequence per tensor. The new token's K/V are computed on-device
(qkv^T projection on the PE) and written into the zero-padded slot the
host leaves at position n_cache of each image: K via a partition-aligned
DVE copy, V via a small SBUF->SBUF DMA from a row-major V_new tile.

Per 128-token tile the compute is 2 PE matmuls (scores = K_T^T @ qpack,
then [ctx^T | denom] += probs^T @ [V | 1]) plus a 1/B share of a
batched exp on the scalar engine — the DVE does only small per-sequence
work, leaving HBM bandwidth as the limiting resource.
"""

import numpy as np
import ml_dtypes

NUM_SEQS = 32
EMBED = 1024
NUM_HEADS = 16
HEAD_DIM = 64
BLOCK_SIZE = 16
N_CORES = 8
HEADS_PER_CORE = NUM_HEADS // N_CORES          # 2
HD = HEADS_PER_CORE * HEAD_DIM                 # 128
SCALE = HEAD_DIM ** -0.5
KDIM = EMBED + 1                               # augmented contraction (bias row)
P = 128
VW = HD + 1                                    # V tile width incl ones column
EXP_BATCH = 8                                  # tiles per batched exp

BF16 = ml_dtypes.bfloat16


def _seq_layout(context_lens):
    """Per-seq (n_tok incl new token, ntiles) + column offsets into images."""
    info = []
    ko = vo = 0
    for s in range(NUM_SEQS):
        n_tok = int(context_lens[s])           # cached tokens + the new one
        nt = (n_tok + P - 1) // P
        info.append((n_tok, nt, ko, vo))
        ko += nt * P
        vo += nt * VW
    return info, ko, vo


def _build_program(context_lens, block_tables):
    import concourse.bass as bass
    import concourse.bacc as bacc
    import concourse.tile as tile
    from concourse import mybir
    from concourse.masks import make_identity

    fp32 = mybir.dt.float32
    bf16 = mybir.dt.bfloat16
    nc = bacc.Bacc("TRN2", target_bir_lowering=False)

    info, KC, VC = _seq_layout(context_lens)

    hTb = nc.declare_dram_parameter("hTb", [KDIM, NUM_SEQS], bf16, isOutput=False)
    wqkvb = nc.declare_dram_parameter("wqkvb", [KDIM, 3 * HD], bf16, isOutput=False)
    wprojb = nc.declare_dram_parameter("wprojb", [HD, EMBED], bf16, isOutput=False)
    kimg = nc.declare_dram_parameter("kimg", [P, KC], bf16, isOutput=False)
    vimg = nc.declare_dram_parameter("vimg", [P, VC], bf16, isOutput=False)
    out_part = nc.declare_dram_parameter("out_part", [NUM_SEQS, EMBED], fp32, isOutput=True)

    with tile.TileContext(nc) as tc:
        with (
            tc.tile_pool(name="persist", bufs=1) as persist,
            tc.tile_pool(name="kpool", bufs=5) as kpool,
            tc.tile_pool(name="vpool", bufs=5) as vpool,
            tc.tile_pool(name="spool", bufs=4) as spool,
            tc.tile_pool(name="prpool", bufs=4) as prpool,
            tc.tile_pool(name="psS", bufs=3, space="PSUM") as psS,
            tc.tile_pool(name="psA", bufs=2, space="PSUM") as psA,
            tc.tile_pool(name="psMisc", bufs=1, space="PSUM") as psMisc,
        ):
            # ---- constants / weights into SBUF ----
            mask01 = persist.tile([P, HEADS_PER_CORE], bf16)
            nc.vector.memset(mask01, 0.0)
            nc.vector.memset(mask01[0:HEAD_DIM, 0:1], 1.0)
            nc.vector.memset(mask01[HEAD_DIM:P, 1:2], 1.0)
            iden2 = persist.tile([HEADS_PER_CORE, HEADS_PER_CORE], bf16)
            make_identity(nc, iden2)

            hT_sb = persist.tile([P, 9, NUM_SEQS], bf16)
            w_sb = persist.tile([P, 9, 3 * HD], bf16)
            for i in range(8):
                nc.sync.dma_start(out=hT_sb[:, i, :], in_=hTb[i * P:(i + 1) * P, :])
                nc.sync.dma_start(out=w_sb[:, i, :], in_=wqkvb[i * P:(i + 1) * P, :])
            nc.sync.dma_start(out=hT_sb[:1, 8, :], in_=hTb[EMBED:KDIM, :])
            nc.sync.dma_start(out=w_sb[:1, 8, :], in_=wqkvb[EMBED:KDIM, :])
            wproj_sb = persist.tile([HD, EMBED], bf16)
            nc.sync.dma_start(out=wproj_sb, in_=wprojb[:, :])

            # ---- qkv^T projection: [128, 32] per part, hd on partitions ----
            qTb = persist.tile([P, NUM_SEQS], bf16)
            kTb = persist.tile([P, NUM_SEQS], bf16)
            for m, dst in enumerate([qTb, kTb]):
                big = psMisc.tile([P, P], fp32, tag="big", bufs=1,
                                  name=f"qkvT_ps{m}")
                for i in range(9):
                    pp = P if i < 8 else 1
                    nc.tensor.matmul(
                        big[:, 0:NUM_SEQS],
                        lhsT=w_sb[:pp, i, m * HD:(m + 1) * HD],
                        rhs=hT_sb[:pp, i, :],
                        start=(i == 0),
                        stop=(i == 8),
                    )
                # fold the attention scale into q
                if m == 0:
                    nc.scalar.mul(dst, big[:, 0:NUM_SEQS], SCALE)
                else:
                    nc.scalar.copy(dst, big[:, 0:NUM_SEQS])
            # V_new row-major [32 seqs, 128 hd]: out = hT^T @ Wv
            vN_sb = persist.tile([NUM_SEQS, HD], bf16)
            big = psMisc.tile([P, P], fp32, tag="big", bufs=1, name="vN_ps")
            for i in range(9):
                pp = P if i < 8 else 1
                nc.tensor.matmul(
                    big[0:NUM_SEQS, :],
                    lhsT=hT_sb[:pp, i, :],
                    rhs=w_sb[:pp, i, 2 * HD:3 * HD],
                    start=(i == 0),
                    stop=(i == 8),
                )
            nc.scalar.copy(vN_sb, big[0:NUM_SEQS, :])

            ctxT_all = persist.tile([P, NUM_SEQS], fp32)

            # ---- per-sequence attention, software-pipelined ----
            # Emission order interleaves stages so the PE never head-of-line
            # blocks: scores for batch j are emitted SKEW batches before the
            # PV matmuls that consume batch j's probs (the exp runs on the
            # scalar engine under the covering scores work), and the small
            # per-seq output transposes are likewise deferred.
            SKEW = 2
            order = sorted(range(NUM_SEQS),
                           key=lambda s: -info[s][1])  # big seqs first
            jobs = []
            for s in order:
                n_tok, nt, ko, vo = info[s]
                units = [(i, min(P, n_tok - i * P)) for i in range(nt)]
                batches = [units[g:g + EXP_BATCH]
                           for g in range(0, len(units), EXP_BATCH)]
                for bi, batch in enumerate(batches):
                    jobs.append((s, bi, batch,
                                 bi == 0, bi == len(batches) - 1))

            seq_state = {}
            job_state = {}
            ct_due = []          # (due_j, s, ctxn)

            def emit_front(j):
                s, bi, batch, first_b, last_b = jobs[j]
                if first_b:
                    n_tok, nt, ko, vo = info[s]
                    k_sb = kpool.tile([P, nt * P], bf16, tag="kimg",
                                      name="k_sb", padded_shape=[P, 32 * P])
                    v_sb = vpool.tile([P, nt * VW], bf16, tag="vimg",
                                      name="v_sb", padded_shape=[P, 32 * VW])
                    nc.sync.dma_start(out=k_sb, in_=kimg[:, ko:ko + nt * P])
                    nc.scalar.dma_start(out=v_sb, in_=vimg[:, vo:vo + nt * VW])
                    # write the new token's K column / V row into the images
                    c = n_tok - 1
                    nc.vector.tensor_copy(k_sb[:, c:c + 1], kTb[:, s:s + 1])
                    ti, r = c // P, c % P
                    nc.gpsimd.dma_start(out=v_sb[r:r + 1, ti * VW:ti * VW + HD],
                                        in_=vN_sb[s:s + 1, :])
                    # qpack [128, 2]: column h = q masked to head h's rows
                    qp = spool.tile([P, HEADS_PER_CORE], bf16, tag="qpack")
                    nc.vector.tensor_mul(
                        qp, qTb[:, s:s + 1].broadcast_to([P, HEADS_PER_CORE]),
                        mask01)
                    acc_ps = psA.tile([HEADS_PER_CORE, VW], fp32, tag="acc",
                                      name="acc_ps")
                    seq_state[s] = (k_sb, v_sb, qp, acc_ps)
                k_sb, v_sb, qp, acc_ps = seq_state[s]
                nb = len(batch)
                s_ps = psS.tile([P, 2 * EXP_BATCH], fp32, tag="scores",
                                name="s_ps")
                for u, (i, n) in enumerate(batch):
                    nc.tensor.matmul(s_ps[0:n, 2 * u:2 * u + 2],
                                     lhsT=k_sb[:, i * P:i * P + n],
                                     rhs=qp, start=True, stop=True)
                pr = prpool.tile([P, 2 * EXP_BATCH], bf16, tag="probs",
                                 name="pr")
                nc.scalar.activation(pr[:, 0:2 * nb], s_ps[:, 0:2 * nb],
                                     mybir.ActivationFunctionType.Exp)
                job_state[j] = pr

            def emit_back(j):
                s, bi, batch, first_b, last_b = jobs[j]
                k_sb, v_sb, qp, acc_ps = seq_state[s]
                pr = job_state.pop(j)
                for u, (i, n) in enumerate(batch):
                    nc.tensor.matmul(acc_ps,
                                     lhsT=pr[0:n, 2 * u:2 * u + 2],
                                     rhs=v_sb[0:n, i * VW:(i + 1) * VW],
                                     start=(first_b and u == 0),
                                     stop=(last_b and u == len(batch) - 1))
                if last_b:
                    # normalize now (DVE, off PE's critical path)
                    rs = spool.tile([HEADS_PER_CORE, 1], fp32, tag="rs",
                                    name="rs")
                    nc.vector.reciprocal(rs, acc_ps[:, HD:VW])
                    ctxn = spool.tile([HEADS_PER_CORE, HD], bf16, tag="ctxn",
                                      name="ctxn")
                    nc.vector.tensor_scalar_mul(ctxn, acc_ps[:, 0:HD], rs)
                    ct_due.append([j + SKEW, s, ctxn])

            def emit_ct(s, ctxn):
                ct_ps = psMisc.tile([P, HEADS_PER_CORE], bf16, tag="ct",
                                    bufs=1, name="ct_ps")
                nc.tensor.transpose(ct_ps, ctxn, iden2)
                nc.vector.tensor_copy(ctxT_all[0:HEAD_DIM, s:s + 1],
                                      ct_ps[0:HEAD_DIM, 0:1])
                nc.vector.tensor_copy(ctxT_all[HEAD_DIM:P, s:s + 1],
                                      ct_ps[HEAD_DIM:P, 1:2])

            for j in range(len(jobs) + SKEW):
                if j < len(jobs):
                    emit_front(j)
                if j >= SKEW:
                    emit_back(j - SKEW)
                for item in list(ct_due):
                    if item[0] <= j:
                        emit_ct(item[1], item[2])
                        ct_due.remove(item)
            for item in ct_due:
                emit_ct(item[1], item[2])

            # ---- c_proj partial: [32, 1024] = ctxT.T @ wproj_slice ----
            ctxTb = persist.tile([P, NUM_SEQS], bf16)
            nc.vector.tensor_copy(ctxTb, ctxT_all)
            out_sb = persist.tile([NUM_SEQS, EMBED], fp32)
            for nblk in range(2):
                cp_ps = psMisc.tile([NUM_SEQS, 512], fp32, tag="cp", bufs=1)
                nc.tensor.matmul(cp_ps, lhsT=ctxTb,
                                 rhs=wproj_sb[:, nblk * 512:(nblk + 1) * 512],
                                 start=True, stop=True)
                nc.vector.tensor_copy(out_sb[:, nblk * 512:(nblk + 1) * 512], cp_ps)
            nc.sync.dma_start(out=out_part[:, :], in_=out_sb)

    nc.finalize()
    return nc


_CACHE = {}


def _prep_inputs(hidden_states, w_attn, b_attn, w_proj, key_cache, value_cache,
                 block_tables, context_lens):
    TOT_SLOTS = key_cache.shape[0] * BLOCK_SIZE
    info, KC, VC = _seq_layout(context_lens)

    hT = np.concatenate([np.ascontiguousarray(hidden_states.T),
                         np.ones((1, NUM_SEQS), np.float32)], axis=0).astype(BF16)
    kc_flat = key_cache.reshape(TOT_SLOTS, NUM_HEADS, HEAD_DIM)
    vc_flat = value_cache.reshape(TOT_SLOTS, NUM_HEADS, HEAD_DIM)

    # token -> physical slot, per sequence, from the block table
    slot_idx = []
    for s in range(NUM_SEQS):
        n_cache = info[s][0] - 1
        t = np.arange(n_cache)
        slot_idx.append(block_tables[s, t // BLOCK_SIZE].astype(np.int64)
                        * BLOCK_SIZE + t % BLOCK_SIZE)

    in_maps = []
    for c in range(N_CORES):
        h0 = c * HEADS_PER_CORE
        cols = []
        for part in range(3):  # q, k, v column blocks of w_attn
            base = part * EMBED + h0 * HEAD_DIM
            cols.append(np.arange(base, base + HD))
        cols = np.concatenate(cols)
        wqkv = np.concatenate([w_attn[:, cols], b_attn[cols][None, :]],
                              axis=0).astype(BF16)
        wproj_c = np.ascontiguousarray(
            w_proj[h0 * HEAD_DIM:(h0 + HEADS_PER_CORE) * HEAD_DIM, :]).astype(BF16)

        kc_c = kc_flat[:, h0:h0 + HEADS_PER_CORE, :].reshape(TOT_SLOTS, HD)
        vc_c = vc_flat[:, h0:h0 + HEADS_PER_CORE, :].reshape(TOT_SLOTS, HD)

        kimg = np.zeros((P, KC), BF16)
        vimg = np.zeros((P, VC), BF16)
        for s in range(NUM_SEQS):
            n_tok, nt, ko, vo = info[s]
            n_cache = n_tok - 1
            ks = np.zeros((nt * P, HD), np.float32)
            vs = np.zeros((nt * P, VW), np.float32)
            ks[:n_cache] = kc_c[slot_idx[s]]
            vs[:n_cache, :HD] = vc_c[slot_idx[s]]
            vs[:, HD] = 1.0                     # pad rows are sliced off
            kimg[:, ko:ko + nt * P] = ks.T.astype(BF16)
            # [nt*P, VW] -> [P, nt, VW]: partition p = token p within tile
            vimg[:, vo:vo + nt * VW] = (
                vs.reshape(nt, P, VW).transpose(1, 0, 2).reshape(P, nt * VW)
            ).astype(BF16)

        in_maps.append({
            "hTb": np.ascontiguousarray(hT),
            "wqkvb": np.ascontiguousarray(wqkv),
            "wprojb": wproj_c,
            "kimg": kimg,
            "vimg": vimg,
        })
    return in_maps


def kernel(hidden_states, w_attn, b_attn, w_proj, b_proj,
           key_cache, value_cache, block_tables, context_lens):
    from concourse.bass_utils import run_bass_kernel_spmd

    import hashlib
    key = hashlib.sha1(np.asarray(context_lens).tobytes()
                       + np.asarray(block_tables).tobytes()).hexdigest()
    if key not in _CACHE:
        _CACHE[key] = _build_program(np.asarray(context_lens), np.asarray(block_tables))
    nc = _CACHE[key]

    in_maps = _prep_inputs(hidden_states, w_attn, b_attn, w_proj,
                           key_cache, value_cache,
                           np.asarray(block_tables), np.asarray(context_lens))
    res = run_bass_kernel_spmd(nc, in_maps, list(range(N_CORES)))
    out = np.zeros((NUM_SEQS, EMBED), np.float32)
    for r in res.results:
        out += r["out_part"]
    out += b_proj[None, :]
    return out


# revision 22
# speedup vs baseline: 435.0222x; 435.0222x over previous
"""Paged-attention GPT-2 decode kernel for 8 Trainium2 NeuronCores.

Sharding: tensor-parallel across heads (Megatron) — 2 heads per core.
Each core gets its head-pair slice of w_attn / w_proj / KV caches and
computes a partial [32,1024] c_proj output; host sums the 8 partials.

The program is specialized to the observed context_lens/block_tables.
Host-side prep packs each sequence's cached K (transposed, [128hd x T])
and V (token-major, with a fused ones-column for the softmax
denominator) into bf16 "SBUF images" so the kernel needs only ONE big
DMA per sequence per tensor. The new token's K/V are computed on-device
(qkv^T projection on the PE) and written into the zero-padded slot the
host leaves at position n_cache of each image: K via a partition-aligned
DVE copy, V via a small SBUF->SBUF DMA from a row-major V_new tile.

Per 128-token tile the compute is 2 PE matmuls (scores = K_T^T @ qpack,
then [ctx^T | denom] += probs^T @ [V | 1]) plus a 1/B share of a
batched exp on the scalar engine — the DVE does only small per-sequence
work, leaving HBM bandwidth as the limiting resource.
"""

import numpy as np
import ml_dtypes

NUM_SEQS = 32
EMBED = 1024
NUM_HEADS = 16
HEAD_DIM = 64
BLOCK_SIZE = 16
N_CORES = 8
HEADS_PER_CORE = NUM_HEADS // N_CORES          # 2
HD = HEADS_PER_CORE * HEAD_DIM                 # 128
SCALE = HEAD_DIM ** -0.5
KDIM = EMBED + 1                               # augmented contraction (bias row)
P = 128
VW = HD + 1                                    # V tile width incl ones column
EXP_BATCH = 8                                  # tiles per batched exp

BF16 = ml_dtypes.bfloat16


def _seq_layout(context_lens):
    """Per-seq (n_tok incl new token, ntiles) + column offsets into images."""
    info = []
    ko = vo = 0
    for s in range(NUM_SEQS):
        n_tok = int(context_lens[s])           # cached tokens + the new one
        nt = (n_tok + P - 1) // P
        info.append((n_tok, nt, ko, vo))
        ko += nt * P
        vo += nt * VW
    return info, ko, vo


def _build_program(context_lens, block_tables):
    import concourse.bass as bass
    import concourse.bacc as bacc
    import concourse.tile as tile
    from concourse import mybir
    from concourse.masks import make_identity

    fp32 = mybir.dt.float32
    bf16 = mybir.dt.bfloat16
    nc = bacc.Bacc("TRN2", target_bir_lowering=False)

    info, KC, VC = _seq_layout(context_lens)

    hTb = nc.declare_dram_parameter("hTb", [KDIM, NUM_SEQS], bf16, isOutput=False)
    wqkvb = nc.declare_dram_parameter("wqkvb", [KDIM, 3 * HD], bf16, isOutput=False)
    wprojb = nc.declare_dram_parameter("wprojb", [HD, EMBED], bf16, isOutput=False)
    kimg = nc.declare_dram_parameter("kimg", [P, KC], bf16, isOutput=False)
    vimg = nc.declare_dram_parameter("vimg", [P, VC], bf16, isOutput=False)
    out_part = nc.declare_dram_parameter("out_part", [NUM_SEQS, EMBED], fp32, isOutput=True)

    with tile.TileContext(nc) as tc:
        with (
            tc.tile_pool(name="persist", bufs=1) as persist,
            tc.tile_pool(name="kpool", bufs=5) as kpool,
            tc.tile_pool(name="vpool", bufs=5) as vpool,
            tc.tile_pool(name="spool", bufs=4) as spool,
            tc.tile_pool(name="prpool", bufs=4) as prpool,
            tc.tile_pool(name="psS", bufs=3, space="PSUM") as psS,
            tc.tile_pool(name="psA", bufs=2, space="PSUM") as psA,
            tc.tile_pool(name="psMisc", bufs=1, space="PSUM") as psMisc,
        ):
            # ---- constants / weights into SBUF ----
            mask01 = persist.tile([P, HEADS_PER_CORE], bf16)
            nc.vector.memset(mask01, 0.0)
            nc.vector.memset(mask01[0:HEAD_DIM, 0:1], 1.0)
            nc.vector.memset(mask01[HEAD_DIM:P, 1:2], 1.0)
            iden2 = persist.tile([HEADS_PER_CORE, HEADS_PER_CORE], bf16)
            make_identity(nc, iden2)

            # weights split across both HWDGE queues, ahead of all KV images,
            # so the qkv^T matmuls aren't starved by image descriptor floods
            hT_sb = persist.tile([P, 9, NUM_SEQS], bf16)
            w_sb = persist.tile([P, 9, 3 * HD], bf16)
            for i in range(8):
                nc.sync.dma_start(out=hT_sb[:, i, :], in_=hTb[i * P:(i + 1) * P, :])
                nc.scalar.dma_start(out=w_sb[:, i, :], in_=wqkvb[i * P:(i + 1) * P, :])
            nc.sync.dma_start(out=hT_sb[:1, 8, :], in_=hTb[EMBED:KDIM, :])
            nc.scalar.dma_start(out=w_sb[:1, 8, :], in_=wqkvb[EMBED:KDIM, :])
            wproj_sb = persist.tile([HD, EMBED], bf16)
            nc.gpsimd.dma_start(out=wproj_sb, in_=wprojb[:, :])

            # ---- qkv^T projection: [128, 32] per part, hd on partitions ----
            qTb = persist.tile([P, NUM_SEQS], bf16)
            kTb = persist.tile([P, NUM_SEQS], bf16)
            for m, dst in enumerate([qTb, kTb]):
                big = psMisc.tile([P, P], fp32, tag="big", bufs=1,
                                  name=f"qkvT_ps{m}")
                for i in range(9):
                    pp = P if i < 8 else 1
                    nc.tensor.matmul(
                        big[:, 0:NUM_SEQS],
                        lhsT=w_sb[:pp, i, m * HD:(m + 1) * HD],
                        rhs=hT_sb[:pp, i, :],
                        start=(i == 0),
                        stop=(i == 8),
                    )
                # fold the attention scale into q
                if m == 0:
                    nc.scalar.mul(dst, big[:, 0:NUM_SEQS], SCALE)
                else:
                    nc.scalar.copy(dst, big[:, 0:NUM_SEQS])
            # V_new row-major [32 seqs, 128 hd]: out = hT^T @ Wv
            vN_sb = persist.tile([NUM_SEQS, HD], bf16)
            big = psMisc.tile([P, P], fp32, tag="big", bufs=1, name="vN_ps")
            for i in range(9):
                pp = P if i < 8 else 1
                nc.tensor.matmul(
                    big[0:NUM_SEQS, :],
                    lhsT=hT_sb[:pp, i, :],
                    rhs=w_sb[:pp, i, 2 * HD:3 * HD],
                    start=(i == 0),
                    stop=(i == 8),
                )
            nc.scalar.copy(vN_sb, big[0:NUM_SEQS, :])

            ctxT_all = persist.tile([P, NUM_SEQS], fp32)

            # ---- per-sequence attention, software-pipelined ----
            # Emission order interleaves stages so the PE never head-of-line
            # blocks: scores for batch j are emitted SKEW batches before the
            # PV matmuls that consume batch j's probs (the exp runs on the
            # scalar engine under the covering scores work), and the small
            # per-seq output transposes are likewise deferred.
            SKEW = 2
            # alternate big/small sequences so the per-seq fixed costs of the
            # small ones overlap with the big ones' DMA streaming instead of
            # bunching into a DMA-starved tail
            desc = sorted(range(NUM_SEQS), key=lambda s: -info[s][1])
            order = []
            lo, hi = 0, NUM_SEQS - 1
            while lo <= hi:
                order.append(desc[lo]); lo += 1
                if lo <= hi:
                    order.append(desc[hi]); hi -= 1
            jobs = []
            for s in order:
                n_tok, nt, ko, vo = info[s]
                units = [(i, min(P, n_tok - i * P)) for i in range(nt)]
                batches = [units[g:g + EXP_BATCH]
                           for g in range(0, len(units), EXP_BATCH)]
                for bi, batch in enumerate(batches):
                    jobs.append((s, bi, batch,
                                 bi == 0, bi == len(batches) - 1))

            seq_state = {}
            job_state = {}
            ct_due = []          # (due_j, s, ctxn)

            def emit_front(j):
                s, bi, batch, first_b, last_b = jobs[j]
                if first_b:
                    n_tok, nt, ko, vo = info[s]
                    k_sb = kpool.tile([P, nt * P], bf16, tag="kimg",
                                      name="k_sb", padded_shape=[P, 32 * P])
                    v_sb = vpool.tile([P, nt * VW], bf16, tag="vimg",
                                      name="v_sb", padded_shape=[P, 32 * VW])
                    nc.sync.dma_start(out=k_sb, in_=kimg[:, ko:ko + nt * P])
                    nc.scalar.dma_start(out=v_sb, in_=vimg[:, vo:vo + nt * VW])
                    # write the new token's K column / V row into the images
                    c = n_tok - 1
                    nc.gpsimd.tensor_copy(k_sb[:, c:c + 1], kTb[:, s:s + 1])
                    ti, r = c // P, c % P
                    nc.gpsimd.dma_start(out=v_sb[r:r + 1, ti * VW:ti * VW + HD],
                                        in_=vN_sb[s:s + 1, :])
                    # qpack [128, 2]: column h = q masked to head h's rows
                    qp = spool.tile([P, HEADS_PER_CORE], bf16, tag="qpack")
                    nc.vector.tensor_mul(
                        qp, qTb[:, s:s + 1].broadcast_to([P, HEADS_PER_CORE]),
                        mask01)
                    acc_ps = psA.tile([HEADS_PER_CORE, VW], fp32, tag="acc",
                                      name="acc_ps")
                    seq_state[s] = (k_sb, v_sb, qp, acc_ps)
                k_sb, v_sb, qp, acc_ps = seq_state[s]
                nb = len(batch)
                s_ps = psS.tile([P, 2 * EXP_BATCH], fp32, tag="scores",
                                name="s_ps")
                for u, (i, n) in enumerate(batch):
                    nc.tensor.matmul(s_ps[0:n, 2 * u:2 * u + 2],
                                     lhsT=k_sb[:, i * P:i * P + n],
                                     rhs=qp, start=True, stop=True)
                pr = prpool.tile([P, 2 * EXP_BATCH], bf16, tag="probs",
                                 name="pr")
                nc.scalar.activation(pr[:, 0:2 * nb], s_ps[:, 0:2 * nb],
                                     mybir.ActivationFunctionType.Exp)
                job_state[j] = pr

            def emit_back(j):
                s, bi, batch, first_b, last_b = jobs[j]
                k_sb, v_sb, qp, acc_ps = seq_state[s]
                pr = job_state.pop(j)
                for u, (i, n) in enumerate(batch):
                    nc.tensor.matmul(acc_ps,
                                     lhsT=pr[0:n, 2 * u:2 * u + 2],
                                     rhs=v_sb[0:n, i * VW:(i + 1) * VW],
                                     start=(first_b and u == 0),
                                     stop=(last_b and u == len(batch) - 1))
                if last_b:
                    # normalize now (DVE, off PE's critical path)
                    rs = spool.tile([HEADS_PER_CORE, 1], fp32, tag="rs",
                                    name="rs")
                    nc.vector.reciprocal(rs, acc_ps[:, HD:VW])
                    ctxn = spool.tile([HEADS_PER_CORE, HD], bf16, tag="ctxn",
                                      name="ctxn")
                    nc.vector.tensor_scalar_mul(ctxn, acc_ps[:, 0:HD], rs)
                    ct_due.append([j + SKEW, s, ctxn])

            def emit_ct(s, ctxn):
                ct_ps = psMisc.tile([P, HEADS_PER_CORE], bf16, tag="ct",
                                    bufs=1, name="ct_ps")
                nc.tensor.transpose(ct_ps, ctxn, iden2)
                nc.vector.tensor_copy(ctxT_all[0:HEAD_DIM, s:s + 1],
                                      ct_ps[0:HEAD_DIM, 0:1])
                nc.vector.tensor_copy(ctxT_all[HEAD_DIM:P, s:s + 1],
                                      ct_ps[HEAD_DIM:P, 1:2])

            for j in range(len(jobs) + SKEW):
                if j < len(jobs):
                    emit_front(j)
                if j >= SKEW:
                    emit_back(j - SKEW)
                for item in list(ct_due):
                    if item[0] <= j:
                        emit_ct(item[1], item[2])
                        ct_due.remove(item)
            for item in ct_due:
                emit_ct(item[1], item[2])

            # ---- c_proj partial: [32, 1024] = ctxT.T @ wproj_slice ----
            ctxTb = persist.tile([P, NUM_SEQS], bf16)
            nc.vector.tensor_copy(ctxTb, ctxT_all)
            out_sb = persist.tile([NUM_SEQS, EMBED], fp32)
            for nblk in range(2):
                cp_ps = psMisc.tile([NUM_SEQS, 512], fp32, tag="cp", bufs=1)
                nc.tensor.matmul(cp_ps, lhsT=ctxTb,
                                 rhs=wproj_sb[:, nblk * 512:(nblk + 1) * 512],
                                 start=True, stop=True)
                nc.vector.tensor_copy(out_sb[:, nblk * 512:(nblk + 1) * 512], cp_ps)
            nc.sync.dma_start(out=out_part[:, :], in_=out_sb)

    nc.finalize()
    return nc


_CACHE = {}


def _prep_inputs(hidden_states, w_attn, b_attn, w_proj, key_cache, value_cache,
                 block_tables, context_lens):
    TOT_SLOTS = key_cache.shape[0] * BLOCK_SIZE
    info, KC, VC = _seq_layout(context_lens)

    hT = np.concatenate([np.ascontiguousarray(hidden_states.T),
                         np.ones((1, NUM_SEQS), np.float32)], axis=0).astype(BF16)
    kc_flat = key_cache.reshape(TOT_SLOTS, NUM_HEADS, HEAD_DIM)
    vc_flat = value_cache.reshape(TOT_SLOTS, NUM_HEADS, HEAD_DIM)

    # token -> physical slot, per sequence, from the block table
    slot_idx = []
    for s in range(NUM_SEQS):
        n_cache = info[s][0] - 1
        t = np.arange(n_cache)
        slot_idx.append(block_tables[s, t // BLOCK_SIZE].astype(np.int64)
                        * BLOCK_SIZE + t % BLOCK_SIZE)

    in_maps = []
    for c in range(N_CORES):
        h0 = c * HEADS_PER_CORE
        cols = []
        for part in range(3):  # q, k, v column blocks of w_attn
            base = part * EMBED + h0 * HEAD_DIM
            cols.append(np.arange(base, base + HD))
        cols = np.concatenate(cols)
        wqkv = np.concatenate([w_attn[:, cols], b_attn[cols][None, :]],
                              axis=0).astype(BF16)
        wproj_c = np.ascontiguousarray(
            w_proj[h0 * HEAD_DIM:(h0 + HEADS_PER_CORE) * HEAD_DIM, :]).astype(BF16)

        kc_c = kc_flat[:, h0:h0 + HEADS_PER_CORE, :].reshape(TOT_SLOTS, HD)
        vc_c = vc_flat[:, h0:h0 + HEADS_PER_CORE, :].reshape(TOT_SLOTS, HD)

        kimg = np.zeros((P, KC), BF16)
        vimg = np.zeros((P, VC), BF16)
        for s in range(NUM_SEQS):
            n_tok, nt, ko, vo = info[s]
            n_cache = n_tok - 1
            ks = np.zeros((nt * P, HD), np.float32)
            vs = np.zeros((nt * P, VW), np.float32)
            ks[:n_cache] = kc_c[slot_idx[s]]
            vs[:n_cache, :HD] = vc_c[slot_idx[s]]
            vs[:, HD] = 1.0                     # pad rows are sliced off
            kimg[:, ko:ko + nt * P] = ks.T.astype(BF16)
            # [nt*P, VW] -> [P, nt, VW]: partition p = token p within tile
            vimg[:, vo:vo + nt * VW] = (
                vs.reshape(nt, P, VW).transpose(1, 0, 2).reshape(P, nt * VW)
            ).astype(BF16)

        in_maps.append({
            "hTb": np.ascontiguousarray(hT),
            "wqkvb": np.ascontiguousarray(wqkv),
            "wprojb": wproj_c,
            "kimg": kimg,
            "vimg": vimg,
        })
    return in_maps


def kernel(hidden_states, w_attn, b_attn, w_proj, b_proj,
           key_cache, value_cache, block_tables, context_lens):
    from concourse.bass_utils import run_bass_kernel_spmd

    import hashlib
    key = hashlib.sha1(np.asarray(context_lens).tobytes()
                       + np.asarray(block_tables).tobytes()).hexdigest()
    if key not in _CACHE:
        _CACHE[key] = _build_program(np.asarray(context_lens), np.asarray(block_tables))
    nc = _CACHE[key]

    in_maps = _prep_inputs(hidden_states, w_attn, b_attn, w_proj,
                           key_cache, value_cache,
                           np.asarray(block_tables), np.asarray(context_lens))
    res = run_bass_kernel_spmd(nc, in_maps, list(range(N_CORES)))
    out = np.zeros((NUM_SEQS, EMBED), np.float32)
    for r in res.results:
        out += r["out_part"]
    out += b_proj[None, :]
    return out
